# revision 41
# baseline (speedup 1.0000x reference)
"""Multi-head causal attention (B=4, S=4096, E=512, H=8) on 8 trn2 NeuronCores.

Sharding: core = (batch b, head-group g of 4 heads); 4 batches x 2 groups = 8 cores.
Each core computes qkv projection for its group's heads, causal attention, and a
partial output projection (its heads' rows of Wo). Host sums the two partials per
batch and adds bo.

All matmul operands are bf16 (fp32/f32r runs the PE in multi-pass mode; bf16
streams 1 col/cycle). PSUM accumulation stays fp32.

The attention loop is software-pipelined to keep the PE continuously busy (the
PE clock drops to half speed after any idle gap, which was the dominant cost):
  - 512-query sweeps; S^T for both heads of a pair lives in ONE [128, 1024]
    PSUM tile (head e cols 0:512, head o 512:1024), double-buffered (4 banks),
    so QK(kb+1) never waits for exp(kb).
  - PV runs one kb behind QK; exp(kb) overlaps QK(kb+1)+PV(kb-1).
  - ov accumulators [65, 512] x2 (2 banks); remaining 2 banks feed interleaved
    projection/Wo "filler" matmuls pulled one group per kb slot.
  - V is stored token-major with a ones column per (kb, head) so the PV matmul
    also produces softmax denominators (row 64 of ov).
"""

import sys

sys.path.insert(0, "/opt/trn_rl_repo")

import numpy as np
import ml_dtypes

BF16 = ml_dtypes.bfloat16

B, S, E = 4, 4096, 512
H = 8
DH = 64
HPG = 4  # heads per group
GQ = 256  # features per group for each of q/k/v (HPG*DH)
QE = 512  # query extent per attention sweep
NQQ = S // QE  # 8
NTQ = 4  # token chunks for projection phase
TQ = S // NTQ  # 1024
VW = HPG * 65  # 260: per-key-block V width incl. ones columns
NEG = -1.0e10
SCALE = 0.125  # 1/sqrt(DH)

_CACHE = {}


def _build_nc():
    import concourse.bass as bass
    import concourse.tile as tile
    import concourse.mybir as mybir
    from concourse import bacc

    f32 = mybir.dt.float32
    bf16 = mybir.dt.bfloat16
    AF = mybir.ActivationFunctionType
    ALU = mybir.AluOpType

    nc = bacc.Bacc("TRN2", target_bir_lowering=False, debug=False)

    xT = nc.dram_tensor("xT", [E, S], bf16, kind="ExternalInput").ap()
    wqk = nc.dram_tensor("wqk", [E, 512], bf16, kind="ExternalInput").ap()
    bqk = nc.dram_tensor("bqk", [128, 4], f32, kind="ExternalInput").ap()
    wv = nc.dram_tensor("wv", [E, GQ], bf16, kind="ExternalInput").ap()
    bv = nc.dram_tensor("bv", [1, GQ], bf16, kind="ExternalInput").ap()
    wo = nc.dram_tensor("wo", [128, 2 * 512], bf16, kind="ExternalInput").ap()
    out = nc.dram_tensor("out", [S, E], f32, kind="ExternalOutput").ap()

    with tile.TileContext(nc) as tc:
        with (
            tc.tile_pool(name="consts", bufs=1) as cpool,
            tc.tile_pool(name="xt", bufs=4) as xtpool,
            tc.tile_pool(name="qkv", bufs=1) as qkvpool,
            tc.tile_pool(name="pt", bufs=3) as ptpool,
            tc.tile_pool(name="att", bufs=1) as attpool,
            tc.tile_pool(name="eps", bufs=2) as epool,
            tc.tile_pool(name="outs", bufs=2) as opool,
            # PSUM: st pair-tiles double-buffered = 4 banks; ov_e+ov_o
            # double-buffered by unit parity = 4 banks. Filler (proj/wo)
            # psum borrows the idle-parity ov banks.
            tc.tile_pool(name="st", bufs=1, space="PSUM") as stpool,
            tc.tile_pool(name="ov", bufs=1, space="PSUM") as ovpool,
        ):
            # ---- constants ----
            wqk_sb = cpool.tile([128, 4 * 512], bf16, name="wqk_sb")
            for ec in range(4):
                nc.sync.dma_start(
                    wqk_sb[:, ec * 512 : (ec + 1) * 512],
                    wqk[ec * 128 : (ec + 1) * 128, :],
                )
            wv_sb = cpool.tile([128, 4 * GQ], bf16, name="wv_sb")
            for ec in range(4):
                nc.sync.dma_start(
                    wv_sb[:, ec * GQ : (ec + 1) * GQ],
                    wv[ec * 128 : (ec + 1) * 128, :],
                )
            wo_sb = cpool.tile([128, 2 * 512], bf16, name="wo_sb")
            nc.sync.dma_start(wo_sb[:], wo[:])
            bqk_sb = cpool.tile([128, 4], f32, name="bqk_sb")
            nc.sync.dma_start(bqk_sb[:], bqk[:])
            bv_sb = cpool.tile([1, GQ], bf16, name="bv_sb")
            nc.sync.dma_start(bv_sb[:], bv[:])
            onesf = cpool.tile([128, 128], f32, name="onesf")
            nc.vector.memset(onesf[:], 1.0)
            ones_row = cpool.tile([1, 128], bf16, name="ones_row")
            nc.vector.tensor_copy(ones_row[:], onesf[0:1, :])
            maskf = cpool.tile([128, 128], f32, name="maskf")
            nc.vector.memset(maskf[:], 0.0)
            nc.gpsimd.affine_select(
                out=maskf[:], in_=maskf[:], compare_op=ALU.is_ge, fill=NEG,
                base=0, pattern=[[1, 128]], channel_multiplier=-1,
            )
            maskT = cpool.tile([128, 128], bf16, name="maskT")
            nc.vector.tensor_copy(maskT[:], maskf[:])
            identf = cpool.tile([128, 128], f32, name="identf")
            nc.vector.memset(identf[:], 0.0)
            nc.gpsimd.affine_select(
                out=identf[:], in_=identf[:], compare_op=ALU.not_equal, fill=1.0,
                base=0, pattern=[[-1, 128]], channel_multiplier=1,
            )
            ident = cpool.tile([128, 128], bf16, name="ident")
            nc.vector.tensor_copy(ident[:], identf[:])

            # persistent qT/kT tiles: [pair A/B][tq] each [128, 1024]
            # pair A rows 0:64 = head0 dh, 64:128 = head1; pair B = heads 2,3
            qt = [
                [qkvpool.tile([128, TQ], bf16, name=f"qt{ab}_{t}") for t in range(NTQ)]
                for ab in range(2)
            ]
            kt = [
                [qkvpool.tile([128, TQ], bf16, name=f"kt{ab}_{t}") for t in range(NTQ)]
                for ab in range(2)
            ]
            vt = [
                qkvpool.tile([128, 8 * VW], bf16, name=f"vt_{t}") for t in range(NTQ)
            ]
            # attention outputs, per 512-query sweep: pair tiles [128, 512]
            # (rows 0:64 head even dh, 64:128 head odd) for K=128 Wo matmuls
            attt = [
                [attpool.tile([128, QE], bf16, name=f"at{q}_{p}") for p in range(2)]
                for q in range(NQQ)
            ]

            # ---- projection / Wo emitted as "filler" groups ----
            # filler psum borrows the ov banks of the currently-idle parity
            borrow = [("ove1", "ovo1")]
            xts_cur = [None]

            def g_xload(tq):
                xts = []
                for ec in range(4):
                    xtile = xtpool.tile([128, TQ], bf16, name="xtile", tag="xtile")
                    nc.sync.dma_start(
                        xtile[:],
                        xT[ec * 128 : (ec + 1) * 128, tq * TQ : (tq + 1) * TQ],
                    )
                    xts.append(xtile)
                xts_cur[0] = xts
                v_tile = vt[tq]
                nc.vector.tensor_copy(
                    v_tile.rearrange("p (t h d) -> p t h d", t=8, h=HPG)[:, :, :, 64:65],
                    onesf[:, 0:32].rearrange("p (t h d) -> p t h d", t=8, h=HPG),
                )

            def g_qk(tq, fc, th):
                xts = xts_cur[0]
                dest = (qt if fc < 2 else kt)[fc % 2][tq]
                ps = ovpool.tile([128, 512], f32, name="fps", tag=borrow[0][th])
                for ec in range(4):
                    nc.tensor.matmul(
                        ps[:],
                        lhsT=wqk_sb[:, ec * 512 + fc * 128 : ec * 512 + (fc + 1) * 128],
                        rhs=xts[ec][:, th * 512 : (th + 1) * 512],
                        start=(ec == 0),
                        stop=(ec == 3),
                    )
                nc.vector.tensor_scalar_add(
                    dest[:, th * 512 : (th + 1) * 512], ps[:], bqk_sb[:, fc : fc + 1]
                )

            def g_v(tq, tb):
                xts = xts_cur[0]
                v_tile = vt[tq]
                vps = ovpool.tile(
                    [128, GQ], f32, name="fvps", tag=borrow[0][tb % 2]
                )
                for ec in range(4):
                    nc.tensor.matmul(
                        vps[:],
                        lhsT=xts[ec][:, tb * 128 : (tb + 1) * 128],
                        rhs=wv_sb[:, ec * GQ : (ec + 1) * GQ],
                        start=(ec == 0),
                        stop=False,
                    )
                nc.tensor.matmul(
                    vps[:], lhsT=ones_row[:], rhs=bv_sb[:], start=False, stop=True
                )
                nc.vector.tensor_copy(
                    v_tile[:, tb * VW : (tb + 1) * VW].rearrange(
                        "p (h d) -> p h d", h=HPG
                    )[:, :, 0:64],
                    vps.rearrange("p (h d) -> p h d", h=HPG),
                )

            osb_cur = [None]

            def g_wo(qq, tb):
                # out rows [qq*1024 + tb*128 ...]: contract both pair tiles
                if tb == 0 or tb == 4:
                    osb_cur[0] = opool.tile(
                        [128, 4 * 512], f32, name="osb", tag=f"osb{qq % 2}"
                    )
                out_sb = osb_cur[0]
                qqp, tbl = (2 * qq + tb // 4), tb % 4
                wops = ovpool.tile(
                    [128, 512], f32, name="fwops", tag=borrow[0][tb % 2]
                )
                for p in range(2):
                    nc.tensor.matmul(
                        wops[:],
                        lhsT=attt[qqp][p][:, tbl * 128 : (tbl + 1) * 128],
                        rhs=wo_sb[:, p * 512 : (p + 1) * 512],
                        start=(p == 0),
                        stop=(p == 1),
                    )
                nc.vector.tensor_copy(out_sb[:, tbl * 512 : (tbl + 1) * 512], wops[:])
                if tbl == 3:
                    half = tb // 4
                    nc.sync.dma_start(
                        out[
                            qq * 1024 + half * 512 : qq * 1024 + (half + 1) * 512, :
                        ].rearrange("(t p) c -> p t c", p=128),
                        out_sb.rearrange("p (t c) -> p t c", t=4),
                    )

            # Filler queues. p1 groups only read DMA'd inputs: safe to pull or
            # force-drain any time. wo groups read att tiles written by
            # epilogues still pending in the PV FIFO: pull-only (never
            # force-drained before the end).
            p1q = []  # (min_sweep, deadline_sweep, fn)
            woq = []  # (min_sweep, fn)

            def p1_groups(tq, min_sweep, deadline):
                p1q.append((min_sweep, deadline, lambda tq=tq: g_xload(tq)))
                for gi, fc in enumerate((0, 2, 1, 3)):
                    for th in range(2):
                        p1q.append(
                            (min_sweep, deadline,
                             lambda tq=tq, fc=fc, th=th: g_qk(tq, fc, th))
                        )
                for tb in range(8):
                    p1q.append(
                        (min_sweep, deadline, lambda tq=tq, tb=tb: g_v(tq, tb))
                    )

            def wo_groups(qq, min_sweep):
                # tb 0-3 read attt[2qq] (ready one sweep earlier than tb 4-7).
                # Defer to sweep >= 5: late sweeps are where the PE runs out of
                # projection filler while ACT paces.
                for tb in range(8):
                    woq.append(
                        (max(min_sweep - (1 if tb < 4 else 0), 5),
                         lambda qq=qq, tb=tb: g_wo(qq, tb))
                    )

            def pull_filler(sweep):
                if p1q and p1q[0][0] <= sweep:
                    p1q.pop(0)[2]()
                elif woq and woq[0][0] <= sweep:
                    woq.pop(0)[1]()

            def drain_p1(sweep):
                while p1q and p1q[0][1] <= sweep:
                    p1q.pop(0)[2]()

            def epilogue(qqp, pr, half, ovt):
                # ovt rows 0:64 = head output [dh, QE], row 64 = softmax sums.
                # Copy out of PSUM first (frees the ov bank for borrowing
                # filler matmuls), then normalize the att tile in place.
                # copy out of PSUM first (frees the ov bank for borrowing
                # filler matmuls); all scratch ops live on partitions 0:64,
                # only the final TT writes to the shifted att rows.
                uat = epool.tile([DH, QE], f32, name="uat", tag="uat")
                nc.vector.tensor_copy(uat[:], ovt[0:DH, :])
                denrow = epool.tile([1, QE], f32, name="denrow", tag="denrow")
                nc.vector.tensor_copy(denrow[:], ovt[64:65, :])
                sbc = epool.tile([DH, QE], f32, name="sbc", tag="sbc")
                nc.sync.dma_start(sbc[:], denrow.unsqueeze(1).to_broadcast([1, DH, QE]))
                rbc = epool.tile([DH, QE], f32, name="rbc", tag="rbc")
                scr = epool.tile([DH, QE], f32, name="scr", tag="rscr", bufs=1)
                nc.vector.reciprocal_approx_accurate(out=rbc[:], in_=sbc[:], scratch=scr[:])
                nc.vector.tensor_tensor(
                    attt[qqp][pr][half * DH : (half + 1) * DH, :],
                    uat[:],
                    rbc[:],
                    ALU.mult,
                )

            # ---- software-pipelined attention ----
            stepc = [0]  # global st-buffer parity
            pend = []  # FIFO of deferred PV emitters; kept 2 slots deep

            def flush_slot(sweep, fill=True):
                if len(pend) > 2:
                    pend.pop(0)()
                    if fill:
                        pull_filler(sweep)

            def flush_all(sweep):
                while pend:
                    pend.pop(0)()

            unitc = [0]  # unit counter: ov parity; filler borrows other parity

            def att_unit(qq, pr):
                nkb = 4 * qq + 4
                par = unitc[0] % 2
                unitc[0] += 1
                borrow[0] = (f"ove{1 - par}", f"ovo{1 - par}")
                ov_e = ovpool.tile([65, QE], f32, name="ov_e", tag=f"ove{par}")
                ov_o = ovpool.tile([65, QE], f32, name="ov_o", tag=f"ovo{par}")
                qtile = qt[pr][qq // 2]
                qoff = (qq % 2) * 512
                for kb in range(nkb):
                    tqk, kbl = kb // 8, kb % 8
                    qs = max(0, (kb - 4 * qq) * 128)
                    st = stpool.tile(
                        [128, 2 * QE], f32, name="st", tag=f"st{stepc[0] % 2}"
                    )
                    stepc[0] += 1
                    for hh in range(2):
                        nc.tensor.matmul(
                            st[:, hh * 512 + qs : hh * 512 + 512],
                            lhsT=kt[pr][tqk][hh * 64 : hh * 64 + 64, kbl * 128 : (kbl + 1) * 128],
                            rhs=qtile[hh * 64 : hh * 64 + 64, qoff + qs : qoff + 512],
                            start=True,
                            stop=True,
                        )
                    if kb >= 4 * qq:  # diagonal: accumulate causal mask on PE
                        for hh in range(2):
                            nc.tensor.matmul(
                                st[:, hh * 512 + qs : hh * 512 + qs + 128],
                                lhsT=ident[:],
                                rhs=maskT[:],
                                start=False,
                                stop=True,
                                skip_group_check=True,
                            )
                    pt = ptpool.tile([128, 2 * QE], bf16, name="pt", tag="pt")
                    if qs == 0 or stepc[0] > 10:
                        # one wide inst; the gap [512, 512+qs) holds stale-but-
                        # bounded psum whose exp lands in pt cols PV never reads
                        nc.scalar.activation(
                            pt[:, qs : 2 * QE], st[:, qs : 2 * QE], AF.Exp,
                            bias=0.0, scale=SCALE,
                        )
                    else:
                        for hh in range(2):
                            nc.scalar.activation(
                                pt[:, hh * 512 + qs : hh * 512 + 512],
                                st[:, hh * 512 + qs : hh * 512 + 512],
                                AF.Exp, bias=0.0, scale=SCALE,
                            )
                    def pv(tqk=tqk, kbl=kbl, qs=qs, kb=kb, pt=pt, pr=pr,
                           ov_e=ov_e, ov_o=ov_o, nkb=nkb):
                        nc.tensor.matmul(
                            ov_e[:, qs:QE],
                            lhsT=vt[tqk][:, kbl * VW + 2 * pr * 65 : kbl * VW + (2 * pr + 1) * 65],
                            rhs=pt[:, qs:512],
                            start=(kb == 0),
                            stop=(kb == nkb - 1),
                            skip_group_check=True,
                        )
                        nc.tensor.matmul(
                            ov_o[:, qs:QE],
                            lhsT=vt[tqk][:, kbl * VW + (2 * pr + 1) * 65 : kbl * VW + (2 * pr + 2) * 65],
                            rhs=pt[:, 512 + qs : 1024],
                            start=(kb == 0),
                            stop=(kb == nkb - 1),
                            skip_group_check=True,
                        )

                    if kb == nkb - 1:
                        def pv_last(pv=pv, qq=qq, pr=pr, ov_e=ov_e, ov_o=ov_o):
                            pv()
                            epilogue(qq, pr, 0, ov_e)
                            epilogue(qq, pr, 1, ov_o)

                        pend.append(pv_last)
                    else:
                        pend.append(pv)
                    fill = (kb >= 2) if qq <= 1 else (kb % 2 == 1 and kb >= 7)
                    flush_slot(qq, fill=fill)

            # ---- schedule ----
            # p1(0) up front; p1(1..3) + wo(0..3) interleaved as filler;
            # wo(3)'s second half at the end.
            p1_groups(0, min_sweep=-1, deadline=0)
            drain_p1(0)
            for tq in range(1, NTQ):
                p1_groups(tq, min_sweep=2 * (tq - 1), deadline=2 * tq)
            wo_groups(0, min_sweep=2)
            wo_groups(1, min_sweep=4)
            wo_groups(2, min_sweep=6)
            wo_groups(3, min_sweep=8)

            for qq in range(NQQ):
                drain_p1(qq)  # p1(tq) must be done before sweep 2tq
                for pr in range(2):
                    att_unit(qq, pr)
            flush_all(NQQ)
            drain_p1(10**9)
            while woq:
                woq.pop(0)[1]()

    nc.finalize()
    return nc


def _get_nc():
    if "nc" not in _CACHE:
        _CACHE["nc"] = _build_nc()
    return _CACHE["nc"]


def _make_in_maps(x, Wqkv, bqkv, Wo):
    in_maps = []
    for core in range(8):
        b, g = core // 2, core % 2
        qs, ks, vs = g * GQ, 512 + g * GQ, 1024 + g * GQ
        wqk_np = np.ascontiguousarray(
            np.concatenate([Wqkv[:, qs : qs + GQ], Wqkv[:, ks : ks + GQ]], axis=1)
        ).astype(BF16)
        bqk_np = np.ascontiguousarray(
            np.concatenate([bqkv[qs : qs + GQ], bqkv[ks : ks + GQ]]).reshape(4, 128).T
        )
        wv_np = np.ascontiguousarray(Wqkv[:, vs : vs + GQ]).astype(BF16)
        bv_np = np.ascontiguousarray(bqkv[vs : vs + GQ].reshape(1, GQ)).astype(BF16)
        wo_g = Wo[g * GQ : (g + 1) * GQ, :]
        # pair p block rows = heads 2p,2p+1 stacked = wo_g[p*128:(p+1)*128]
        wo_np = np.ascontiguousarray(
            np.concatenate([wo_g[0:128, :], wo_g[128:256, :]], axis=1)
        ).astype(BF16)
        in_maps.append(
            {
                "xT": np.ascontiguousarray(x[b].T).astype(BF16),
                "wqk": wqk_np,
                "bqk": bqk_np,
                "wv": wv_np,
                "bv": bv_np,
                "wo": wo_np,
            }
        )
    return in_maps


def kernel(x, Wqkv, bqkv, Wo, bo, **run_kwargs):
    from concourse.bass_utils import run_bass_kernel_spmd

    x = np.asarray(x, dtype=np.float32)
    Wqkv = np.asarray(Wqkv, dtype=np.float32)
    bqkv = np.asarray(bqkv, dtype=np.float32)
    Wo = np.asarray(Wo, dtype=np.float32)
    bo = np.asarray(bo, dtype=np.float32)

    nc = _get_nc()
    in_maps = _make_in_maps(x, Wqkv, bqkv, Wo)

    res = run_bass_kernel_spmd(nc, in_maps, core_ids=list(range(8)), **run_kwargs)
    _CACHE["last_results"] = res

    out = np.empty((B, S, E), dtype=np.float32)
    for b in range(B):
        out[b] = res.results[2 * b]["out"] + res.results[2 * b + 1]["out"] + bo
    return out


# revision 43
# speedup vs baseline: 1.0322x; 1.0322x over previous
"""Multi-head causal attention (B=4, S=4096, E=512, H=8) on 8 trn2 NeuronCores.

Sharding: core = (batch b, head-group g of 4 heads); 4 batches x 2 groups = 8 cores.
Each core computes qkv projection for its group's heads, causal attention, and a
partial output projection (its heads' rows of Wo). Host sums the two partials per
batch and adds bo.

All matmul operands are bf16 (fp32/f32r runs the PE in multi-pass mode; bf16
streams 1 col/cycle). PSUM accumulation stays fp32.

The attention loop is software-pipelined to keep the PE continuously busy (the
PE clock drops to half speed after any idle gap, which was the dominant cost):
  - 512-query sweeps; S^T for both heads of a pair lives in ONE [128, 1024]
    PSUM tile (head e cols 0:512, head o 512:1024), double-buffered (4 banks),
    so QK(kb+1) never waits for exp(kb).
  - PV runs one kb behind QK; exp(kb) overlaps QK(kb+1)+PV(kb-1).
  - ov accumulators [65, 512] x2 (2 banks); remaining 2 banks feed interleaved
    projection/Wo "filler" matmuls pulled one group per kb slot.
  - V is stored token-major with a ones column per (kb, head) so the PV matmul
    also produces softmax denominators (row 64 of ov).
"""

import sys

sys.path.insert(0, "/opt/trn_rl_repo")

import numpy as np
import ml_dtypes

BF16 = ml_dtypes.bfloat16

B, S, E = 4, 4096, 512
H = 8
DH = 64
HPG = 4  # heads per group
GQ = 256  # features per group for each of q/k/v (HPG*DH)
QE = 512  # query extent per attention sweep
NQQ = S // QE  # 8
NTQ = 4  # token chunks for projection phase
TQ = S // NTQ  # 1024
VW = HPG * 65  # 260: per-key-block V width incl. ones columns
NEG = -1.0e10
SCALE = 0.125  # 1/sqrt(DH)

_CACHE = {}


def _build_nc():
    import concourse.bass as bass
    import concourse.tile as tile
    import concourse.mybir as mybir
    from concourse import bacc

    f32 = mybir.dt.float32
    bf16 = mybir.dt.bfloat16
    AF = mybir.ActivationFunctionType
    ALU = mybir.AluOpType

    nc = bacc.Bacc("TRN2", target_bir_lowering=False, debug=False)

    xT = nc.dram_tensor("xT", [E, S], bf16, kind="ExternalInput").ap()
    wqk = nc.dram_tensor("wqk", [E, 512], bf16, kind="ExternalInput").ap()
    bqk = nc.dram_tensor("bqk", [128, 4], f32, kind="ExternalInput").ap()
    wv = nc.dram_tensor("wv", [E, GQ], bf16, kind="ExternalInput").ap()
    bv = nc.dram_tensor("bv", [1, GQ], bf16, kind="ExternalInput").ap()
    wo = nc.dram_tensor("wo", [128, 2 * 512], bf16, kind="ExternalInput").ap()
    out = nc.dram_tensor("out", [S, E], f32, kind="ExternalOutput").ap()

    with tile.TileContext(nc) as tc:
        with (
            tc.tile_pool(name="consts", bufs=1) as cpool,
            tc.tile_pool(name="xt", bufs=4) as xtpool,
            tc.tile_pool(name="qkv", bufs=1) as qkvpool,
            tc.tile_pool(name="pt", bufs=3) as ptpool,
            tc.tile_pool(name="att", bufs=1) as attpool,
            tc.tile_pool(name="eps", bufs=2) as epool,
            tc.tile_pool(name="outs", bufs=2) as opool,
            # PSUM: st pair-tiles double-buffered = 4 banks; ov_e+ov_o
            # double-buffered by unit parity = 4 banks. Filler (proj/wo)
            # psum borrows the idle-parity ov banks.
            tc.tile_pool(name="st", bufs=1, space="PSUM") as stpool,
            tc.tile_pool(name="ov", bufs=1, space="PSUM") as ovpool,
        ):
            # ---- constants ----
            wqk_sb = cpool.tile([128, 4 * 512], bf16, name="wqk_sb")
            for ec in range(4):
                nc.sync.dma_start(
                    wqk_sb[:, ec * 512 : (ec + 1) * 512],
                    wqk[ec * 128 : (ec + 1) * 128, :],
                )
            wv_sb = cpool.tile([128, 4 * GQ], bf16, name="wv_sb")
            for ec in range(4):
                nc.sync.dma_start(
                    wv_sb[:, ec * GQ : (ec + 1) * GQ],
                    wv[ec * 128 : (ec + 1) * 128, :],
                )
            wo_sb = cpool.tile([128, 2 * 512], bf16, name="wo_sb")
            nc.sync.dma_start(wo_sb[:], wo[:])
            bqk_sb = cpool.tile([128, 4], f32, name="bqk_sb")
            nc.sync.dma_start(bqk_sb[:], bqk[:])
            bv_sb = cpool.tile([1, GQ], bf16, name="bv_sb")
            nc.sync.dma_start(bv_sb[:], bv[:])
            onesf = cpool.tile([128, 128], f32, name="onesf")
            nc.vector.memset(onesf[:], 1.0)
            ones_row = cpool.tile([1, 128], bf16, name="ones_row")
            nc.vector.tensor_copy(ones_row[:], onesf[0:1, :])
            maskf = cpool.tile([128, 128], f32, name="maskf")
            nc.vector.memset(maskf[:], 0.0)
            nc.gpsimd.affine_select(
                out=maskf[:], in_=maskf[:], compare_op=ALU.is_ge, fill=NEG,
                base=0, pattern=[[1, 128]], channel_multiplier=-1,
            )
            maskT = cpool.tile([128, 128], bf16, name="maskT")
            nc.vector.tensor_copy(maskT[:], maskf[:])
            identf = cpool.tile([128, 128], f32, name="identf")
            nc.vector.memset(identf[:], 0.0)
            nc.gpsimd.affine_select(
                out=identf[:], in_=identf[:], compare_op=ALU.not_equal, fill=1.0,
                base=0, pattern=[[-1, 128]], channel_multiplier=1,
            )
            ident = cpool.tile([128, 128], bf16, name="ident")
            nc.vector.tensor_copy(ident[:], identf[:])

            # persistent qT/kT tiles: [pair A/B][tq] each [128, 1024]
            # pair A rows 0:64 = head0 dh, 64:128 = head1; pair B = heads 2,3
            qt = [
                [qkvpool.tile([128, TQ], bf16, name=f"qt{ab}_{t}") for t in range(NTQ)]
                for ab in range(2)
            ]
            kt = [
                [qkvpool.tile([128, TQ], bf16, name=f"kt{ab}_{t}") for t in range(NTQ)]
                for ab in range(2)
            ]
            vt = [
                qkvpool.tile([128, 8 * VW], bf16, name=f"vt_{t}") for t in range(NTQ)
            ]
            # attention outputs, per 512-query sweep: pair tiles [128, 512]
            # (rows 0:64 head even dh, 64:128 head odd) for K=128 Wo matmuls
            attt = [
                [attpool.tile([128, QE], bf16, name=f"at{q}_{p}") for p in range(2)]
                for q in range(NQQ)
            ]

            # ---- projection / Wo emitted as "filler" groups ----
            # filler psum borrows the ov banks of the currently-idle parity
            borrow = [("ove1", "ovo1")]
            xts_cur = [None]

            def g_xload(tq):
                xts = []
                for ec in range(4):
                    xtile = xtpool.tile([128, TQ], bf16, name="xtile", tag="xtile")
                    nc.sync.dma_start(
                        xtile[:],
                        xT[ec * 128 : (ec + 1) * 128, tq * TQ : (tq + 1) * TQ],
                    )
                    xts.append(xtile)
                xts_cur[0] = xts
                v_tile = vt[tq]
                nc.vector.tensor_copy(
                    v_tile.rearrange("p (t h d) -> p t h d", t=8, h=HPG)[:, :, :, 64:65],
                    onesf[:, 0:32].rearrange("p (t h d) -> p t h d", t=8, h=HPG),
                )

            def g_qk(tq, fc, th):
                xts = xts_cur[0]
                dest = (qt if fc < 2 else kt)[fc % 2][tq]
                ps = ovpool.tile([128, 512], f32, name="fps", tag=borrow[0][th])
                for ec in range(4):
                    nc.tensor.matmul(
                        ps[:],
                        lhsT=wqk_sb[:, ec * 512 + fc * 128 : ec * 512 + (fc + 1) * 128],
                        rhs=xts[ec][:, th * 512 : (th + 1) * 512],
                        start=(ec == 0),
                        stop=(ec == 3),
                    )
                nc.vector.tensor_scalar_add(
                    dest[:, th * 512 : (th + 1) * 512], ps[:], bqk_sb[:, fc : fc + 1]
                )

            def g_v(tq, tb):
                xts = xts_cur[0]
                v_tile = vt[tq]
                vps = ovpool.tile(
                    [128, GQ], f32, name="fvps", tag=borrow[0][tb % 2]
                )
                for ec in range(4):
                    nc.tensor.matmul(
                        vps[:],
                        lhsT=xts[ec][:, tb * 128 : (tb + 1) * 128],
                        rhs=wv_sb[:, ec * GQ : (ec + 1) * GQ],
                        start=(ec == 0),
                        stop=False,
                    )
                nc.tensor.matmul(
                    vps[:], lhsT=ones_row[:], rhs=bv_sb[:], start=False, stop=True
                )
                nc.vector.tensor_copy(
                    v_tile[:, tb * VW : (tb + 1) * VW].rearrange(
                        "p (h d) -> p h d", h=HPG
                    )[:, :, 0:64],
                    vps.rearrange("p (h d) -> p h d", h=HPG),
                )

            osb_cur = [None]

            def g_wo(qq, tb):
                # out rows [qq*1024 + tb*128 ...]: contract both pair tiles
                if tb == 0 or tb == 4:
                    osb_cur[0] = opool.tile(
                        [128, 4 * 512], f32, name="osb", tag=f"osb{qq % 2}"
                    )
                out_sb = osb_cur[0]
                qqp, tbl = (2 * qq + tb // 4), tb % 4
                wops = ovpool.tile(
                    [128, 512], f32, name="fwops", tag=borrow[0][tb % 2]
                )
                for p in range(2):
                    nc.tensor.matmul(
                        wops[:],
                        lhsT=attt[qqp][p][:, tbl * 128 : (tbl + 1) * 128],
                        rhs=wo_sb[:, p * 512 : (p + 1) * 512],
                        start=(p == 0),
                        stop=(p == 1),
                    )
                nc.vector.tensor_copy(out_sb[:, tbl * 512 : (tbl + 1) * 512], wops[:])
                if tbl == 3:
                    half = tb // 4
                    nc.sync.dma_start(
                        out[
                            qq * 1024 + half * 512 : qq * 1024 + (half + 1) * 512, :
                        ].rearrange("(t p) c -> p t c", p=128),
                        out_sb.rearrange("p (t c) -> p t c", t=4),
                    )

            # Filler queues. p1 groups only read DMA'd inputs: safe to pull or
            # force-drain any time. wo groups read att tiles written by
            # epilogues still pending in the PV FIFO: pull-only (never
            # force-drained before the end).
            p1q = []  # (min_sweep, deadline_sweep, fn)
            woq = []  # (min_sweep, fn)

            def p1_groups(tq, min_sweep, deadline):
                p1q.append((min_sweep, deadline, lambda tq=tq: g_xload(tq)))
                for gi, fc in enumerate((0, 2, 1, 3)):
                    for th in range(2):
                        p1q.append(
                            (min_sweep, deadline,
                             lambda tq=tq, fc=fc, th=th: g_qk(tq, fc, th))
                        )
                for tb in range(8):
                    p1q.append(
                        (min_sweep, deadline, lambda tq=tq, tb=tb: g_v(tq, tb))
                    )

            def wo_groups(qq, min_sweep):
                # tb 0-3 read attt[2qq] (ready one sweep earlier than tb 4-7).
                # Defer to sweep >= 5: late sweeps are where the PE runs out of
                # projection filler while ACT paces.
                for tb in range(8):
                    woq.append(
                        (max(min_sweep - (1 if tb < 4 else 0), 5),
                         lambda qq=qq, tb=tb: g_wo(qq, tb))
                    )

            def pull_filler(sweep):
                if p1q and p1q[0][0] <= sweep:
                    p1q.pop(0)[2]()
                elif woq and woq[0][0] <= sweep:
                    woq.pop(0)[1]()

            def drain_p1(sweep):
                while p1q and p1q[0][1] <= sweep:
                    p1q.pop(0)[2]()

            def epilogue(qqp, pr, ov_e, ov_o):
                # ov rows 0:64 = head output [dh, QE], row 64 = softmax sums.
                # Copy out of PSUM first (frees the ov banks), then one
                # broadcast DMA + one reciprocal serve both heads.
                pair = attt[qqp][pr]
                uat = epool.tile([128, QE], f32, name="uat", tag="uat")
                nc.vector.tensor_copy(uat[0:DH, :], ov_e[0:DH, :])
                nc.vector.tensor_copy(uat[DH:128, :], ov_o[0:DH, :])
                den2 = epool.tile([128, QE], f32, name="den2", tag="den2")
                nc.vector.tensor_copy(den2[0:1, :], ov_e[64:65, :])
                nc.vector.tensor_copy(den2[DH : DH + 1, :], ov_o[64:65, :])
                sbc = epool.tile([128, QE], f32, name="sbc", tag="sbc")
                nc.sync.dma_start(
                    sbc[:],
                    den2.rearrange("(a b) c -> a b c", a=2)[:, 0:1, :].to_broadcast(
                        [2, DH, QE]
                    ),
                )
                rbc = epool.tile([128, QE], f32, name="rbc", tag="rbc")
                nc.vector.reciprocal_approx_fast(out=rbc[:], in_=sbc[:])
                nc.vector.tensor_tensor(pair[0:DH, :], uat[0:DH, :], rbc[0:DH, :], ALU.mult)
                nc.vector.tensor_tensor(
                    pair[DH:128, :], uat[DH:128, :], rbc[DH:128, :], ALU.mult
                )

            # ---- software-pipelined attention ----
            stepc = [0]  # global st-buffer parity
            pend = []  # FIFO of deferred PV emitters; kept 2 slots deep

            def flush_slot(sweep, fill=True):
                if len(pend) > 2:
                    pend.pop(0)()
                    if fill:
                        pull_filler(sweep)

            def flush_all(sweep):
                while pend:
                    pend.pop(0)()

            unitc = [0]  # unit counter: ov parity; filler borrows other parity

            def att_unit(qq, pr):
                nkb = 4 * qq + 4
                par = unitc[0] % 2
                unitc[0] += 1
                borrow[0] = (f"ove{1 - par}", f"ovo{1 - par}")
                ov_e = ovpool.tile([65, QE], f32, name="ov_e", tag=f"ove{par}")
                ov_o = ovpool.tile([65, QE], f32, name="ov_o", tag=f"ovo{par}")
                qtile = qt[pr][qq // 2]
                qoff = (qq % 2) * 512
                for kb in range(nkb):
                    tqk, kbl = kb // 8, kb % 8
                    qs = max(0, (kb - 4 * qq) * 128)
                    st = stpool.tile(
                        [128, 2 * QE], f32, name="st", tag=f"st{stepc[0] % 2}"
                    )
                    stepc[0] += 1
                    for hh in range(2):
                        nc.tensor.matmul(
                            st[:, hh * 512 + qs : hh * 512 + 512],
                            lhsT=kt[pr][tqk][hh * 64 : hh * 64 + 64, kbl * 128 : (kbl + 1) * 128],
                            rhs=qtile[hh * 64 : hh * 64 + 64, qoff + qs : qoff + 512],
                            start=True,
                            stop=True,
                        )
                    if kb >= 4 * qq:  # diagonal: accumulate causal mask on PE
                        for hh in range(2):
                            nc.tensor.matmul(
                                st[:, hh * 512 + qs : hh * 512 + qs + 128],
                                lhsT=ident[:],
                                rhs=maskT[:],
                                start=False,
                                stop=True,
                                skip_group_check=True,
                            )
                    pt = ptpool.tile([128, 2 * QE], bf16, name="pt", tag="pt")
                    if qs == 0 or stepc[0] > 10:
                        # one wide inst; the gap [512, 512+qs) holds stale-but-
                        # bounded psum whose exp lands in pt cols PV never reads
                        nc.scalar.activation(
                            pt[:, qs : 2 * QE], st[:, qs : 2 * QE], AF.Exp,
                            bias=0.0, scale=SCALE,
                        )
                    else:
                        for hh in range(2):
                            nc.scalar.activation(
                                pt[:, hh * 512 + qs : hh * 512 + 512],
                                st[:, hh * 512 + qs : hh * 512 + 512],
                                AF.Exp, bias=0.0, scale=SCALE,
                            )
                    def pv(tqk=tqk, kbl=kbl, qs=qs, kb=kb, pt=pt, pr=pr,
                           ov_e=ov_e, ov_o=ov_o, nkb=nkb):
                        nc.tensor.matmul(
                            ov_e[:, qs:QE],
                            lhsT=vt[tqk][:, kbl * VW + 2 * pr * 65 : kbl * VW + (2 * pr + 1) * 65],
                            rhs=pt[:, qs:512],
                            start=(kb == 0),
                            stop=(kb == nkb - 1),
                            skip_group_check=True,
                        )
                        nc.tensor.matmul(
                            ov_o[:, qs:QE],
                            lhsT=vt[tqk][:, kbl * VW + (2 * pr + 1) * 65 : kbl * VW + (2 * pr + 2) * 65],
                            rhs=pt[:, 512 + qs : 1024],
                            start=(kb == 0),
                            stop=(kb == nkb - 1),
                            skip_group_check=True,
                        )

                    if kb == nkb - 1:
                        def pv_last(pv=pv, qq=qq, pr=pr, ov_e=ov_e, ov_o=ov_o):
                            pv()
                            epilogue(qq, pr, ov_e, ov_o)

                        pend.append(pv_last)
                    else:
                        pend.append(pv)
                    fill = (
                        (2 <= kb < nkb - 1) if qq <= 1
                        else (kb % 2 == 1 and 7 <= kb < nkb - 2)
                    )
                    flush_slot(qq, fill=fill)

            # ---- schedule ----
            # p1(0) up front; p1(1..3) + wo(0..3) interleaved as filler;
            # wo(3)'s second half at the end.
            p1_groups(0, min_sweep=-1, deadline=0)
            drain_p1(0)
            for tq in range(1, NTQ):
                p1_groups(tq, min_sweep=2 * (tq - 1), deadline=2 * tq)
            wo_groups(0, min_sweep=2)
            wo_groups(1, min_sweep=4)
            wo_groups(2, min_sweep=6)
            wo_groups(3, min_sweep=8)

            for qq in range(NQQ):
                drain_p1(qq)  # p1(tq) must be done before sweep 2tq
                for pr in range(2):
                    att_unit(qq, pr)
            flush_all(NQQ)
            drain_p1(10**9)
            while woq:
                woq.pop(0)[1]()

    nc.finalize()
    return nc


def _get_nc():
    if "nc" not in _CACHE:
        _CACHE["nc"] = _build_nc()
    return _CACHE["nc"]


def _make_in_maps(x, Wqkv, bqkv, Wo):
    in_maps = []
    for core in range(8):
        b, g = core // 2, core % 2
        qs, ks, vs = g * GQ, 512 + g * GQ, 1024 + g * GQ
        wqk_np = np.ascontiguousarray(
            np.concatenate([Wqkv[:, qs : qs + GQ], Wqkv[:, ks : ks + GQ]], axis=1)
        ).astype(BF16)
        bqk_np = np.ascontiguousarray(
            np.concatenate([bqkv[qs : qs + GQ], bqkv[ks : ks + GQ]]).reshape(4, 128).T
        )
        wv_np = np.ascontiguousarray(Wqkv[:, vs : vs + GQ]).astype(BF16)
        bv_np = np.ascontiguousarray(bqkv[vs : vs + GQ].reshape(1, GQ)).astype(BF16)
        wo_g = Wo[g * GQ : (g + 1) * GQ, :]
        # pair p block rows = heads 2p,2p+1 stacked = wo_g[p*128:(p+1)*128]
        wo_np = np.ascontiguousarray(
            np.concatenate([wo_g[0:128, :], wo_g[128:256, :]], axis=1)
        ).astype(BF16)
        in_maps.append(
            {
                "xT": np.ascontiguousarray(x[b].T).astype(BF16),
                "wqk": wqk_np,
                "bqk": bqk_np,
                "wv": wv_np,
                "bv": bv_np,
                "wo": wo_np,
            }
        )
    return in_maps


def kernel(x, Wqkv, bqkv, Wo, bo, **run_kwargs):
    from concourse.bass_utils import run_bass_kernel_spmd

    x = np.asarray(x, dtype=np.float32)
    Wqkv = np.asarray(Wqkv, dtype=np.float32)
    bqkv = np.asarray(bqkv, dtype=np.float32)
    Wo = np.asarray(Wo, dtype=np.float32)
    bo = np.asarray(bo, dtype=np.float32)

    nc = _get_nc()
    in_maps = _make_in_maps(x, Wqkv, bqkv, Wo)

    res = run_bass_kernel_spmd(nc, in_maps, core_ids=list(range(8)), **run_kwargs)
    _CACHE["last_results"] = res

    out = np.empty((B, S, E), dtype=np.float32)
    for b in range(B):
        out[b] = res.results[2 * b]["out"] + res.results[2 * b + 1]["out"] + bo
    return out


# revision 44
# speedup vs baseline: 1.0758x; 1.0422x over previous
"""Multi-head causal attention (B=4, S=4096, E=512, H=8) on 8 trn2 NeuronCores.

Sharding: core = (batch b, head-group g of 4 heads); 4 batches x 2 groups = 8 cores.
Each core computes qkv projection for its group's heads, causal attention, and a
partial output projection (its heads' rows of Wo). Host sums the two partials per
batch and adds bo.

All matmul operands are bf16 (fp32/f32r runs the PE in multi-pass mode; bf16
streams 1 col/cycle). PSUM accumulation stays fp32.

The attention loop is software-pipelined to keep the PE continuously busy (the
PE clock drops to half speed after any idle gap, which was the dominant cost):
  - 512-query sweeps; S^T for both heads of a pair lives in ONE [128, 1024]
    PSUM tile (head e cols 0:512, head o 512:1024), double-buffered (4 banks),
    so QK(kb+1) never waits for exp(kb).
  - PV runs one kb behind QK; exp(kb) overlaps QK(kb+1)+PV(kb-1).
  - ov accumulators [65, 512] x2 (2 banks); remaining 2 banks feed interleaved
    projection/Wo "filler" matmuls pulled one group per kb slot.
  - V is stored token-major with a ones column per (kb, head) so the PV matmul
    also produces softmax denominators (row 64 of ov).
"""

import sys

sys.path.insert(0, "/opt/trn_rl_repo")

import numpy as np
import ml_dtypes

BF16 = ml_dtypes.bfloat16

B, S, E = 4, 4096, 512
H = 8
DH = 64
HPG = 4  # heads per group
GQ = 256  # features per group for each of q/k/v (HPG*DH)
QE = 512  # query extent per attention sweep
NQQ = S // QE  # 8
NTQ = 4  # token chunks for projection phase
TQ = S // NTQ  # 1024
VW = HPG * 65  # 260: per-key-block V width incl. ones columns
NEG = -1.0e10
SCALE = 0.125  # 1/sqrt(DH)

_CACHE = {}


def _build_nc():
    import concourse.bass as bass
    import concourse.tile as tile
    import concourse.mybir as mybir
    from concourse import bacc

    f32 = mybir.dt.float32
    bf16 = mybir.dt.bfloat16
    AF = mybir.ActivationFunctionType
    ALU = mybir.AluOpType

    nc = bacc.Bacc("TRN2", target_bir_lowering=False, debug=False)

    xT = nc.dram_tensor("xT", [E, S], bf16, kind="ExternalInput").ap()
    wqk = nc.dram_tensor("wqk", [E, 512], bf16, kind="ExternalInput").ap()
    bqk = nc.dram_tensor("bqk", [128, 4], f32, kind="ExternalInput").ap()
    wv = nc.dram_tensor("wv", [E, GQ], bf16, kind="ExternalInput").ap()
    bv = nc.dram_tensor("bv", [1, GQ], bf16, kind="ExternalInput").ap()
    wo = nc.dram_tensor("wo", [128, 2 * 512], bf16, kind="ExternalInput").ap()
    out = nc.dram_tensor("out", [S, E], f32, kind="ExternalOutput").ap()

    with tile.TileContext(nc) as tc:
        with (
            tc.tile_pool(name="consts", bufs=1) as cpool,
            tc.tile_pool(name="xt", bufs=4) as xtpool,
            tc.tile_pool(name="qkv", bufs=1) as qkvpool,
            tc.tile_pool(name="pt", bufs=3) as ptpool,
            tc.tile_pool(name="att", bufs=1) as attpool,
            tc.tile_pool(name="eps", bufs=2) as epool,
            tc.tile_pool(name="outs", bufs=2) as opool,
            # PSUM: st pair-tiles double-buffered = 4 banks; ov_e+ov_o
            # double-buffered by unit parity = 4 banks. Filler (proj/wo)
            # psum borrows the idle-parity ov banks.
            tc.tile_pool(name="st", bufs=1, space="PSUM") as stpool,
            tc.tile_pool(name="ov", bufs=1, space="PSUM") as ovpool,
        ):
            # ---- constants ----
            wqk_sb = cpool.tile([128, 4 * 512], bf16, name="wqk_sb")
            for ec in range(4):
                nc.sync.dma_start(
                    wqk_sb[:, ec * 512 : (ec + 1) * 512],
                    wqk[ec * 128 : (ec + 1) * 128, :],
                )
            wv_sb = cpool.tile([128, 4 * GQ], bf16, name="wv_sb")
            for ec in range(4):
                nc.sync.dma_start(
                    wv_sb[:, ec * GQ : (ec + 1) * GQ],
                    wv[ec * 128 : (ec + 1) * 128, :],
                )
            wo_sb = cpool.tile([128, 2 * 512], bf16, name="wo_sb")
            nc.sync.dma_start(wo_sb[:], wo[:])
            bqk_sb = cpool.tile([128, 4], f32, name="bqk_sb")
            nc.sync.dma_start(bqk_sb[:], bqk[:])
            bv_sb = cpool.tile([1, GQ], bf16, name="bv_sb")
            nc.sync.dma_start(bv_sb[:], bv[:])
            onesf = cpool.tile([128, 128], f32, name="onesf")
            nc.vector.memset(onesf[:], 1.0)
            ones_row = cpool.tile([1, 128], bf16, name="ones_row")
            nc.vector.tensor_copy(ones_row[:], onesf[0:1, :])
            maskf = cpool.tile([128, 128], f32, name="maskf")
            nc.vector.memset(maskf[:], 0.0)
            nc.gpsimd.affine_select(
                out=maskf[:], in_=maskf[:], compare_op=ALU.is_ge, fill=NEG,
                base=0, pattern=[[1, 128]], channel_multiplier=-1,
            )
            maskT = cpool.tile([128, 128], bf16, name="maskT")
            nc.vector.tensor_copy(maskT[:], maskf[:])
            identf = cpool.tile([128, 128], f32, name="identf")
            nc.vector.memset(identf[:], 0.0)
            nc.gpsimd.affine_select(
                out=identf[:], in_=identf[:], compare_op=ALU.not_equal, fill=1.0,
                base=0, pattern=[[-1, 128]], channel_multiplier=1,
            )
            ident = cpool.tile([128, 128], bf16, name="ident")
            nc.vector.tensor_copy(ident[:], identf[:])

            # persistent qT/kT tiles: [pair A/B][tq] each [128, 1024]
            # pair A rows 0:64 = head0 dh, 64:128 = head1; pair B = heads 2,3
            qt = [
                [qkvpool.tile([128, TQ], bf16, name=f"qt{ab}_{t}") for t in range(NTQ)]
                for ab in range(2)
            ]
            kt = [
                [qkvpool.tile([128, TQ], bf16, name=f"kt{ab}_{t}") for t in range(NTQ)]
                for ab in range(2)
            ]
            vt = [
                qkvpool.tile([128, 8 * VW], bf16, name=f"vt_{t}") for t in range(NTQ)
            ]
            # attention outputs, per 512-query sweep: pair tiles [128, 512]
            # (rows 0:64 head even dh, 64:128 head odd) for K=128 Wo matmuls
            attt = [
                [attpool.tile([128, QE], bf16, name=f"at{q}_{p}") for p in range(2)]
                for q in range(NQQ)
            ]

            # ---- projection / Wo emitted as "filler" groups ----
            # filler psum borrows the ov banks of the currently-idle parity
            borrow = [("ove1", "ovo1")]
            xts_cur = [None]

            def g_xload(tq):
                xts = []
                for ec in range(4):
                    xtile = xtpool.tile([128, TQ], bf16, name="xtile", tag="xtile")
                    nc.sync.dma_start(
                        xtile[:],
                        xT[ec * 128 : (ec + 1) * 128, tq * TQ : (tq + 1) * TQ],
                    )
                    xts.append(xtile)
                xts_cur[0] = xts
                v_tile = vt[tq]
                nc.vector.tensor_copy(
                    v_tile.rearrange("p (t h d) -> p t h d", t=8, h=HPG)[:, :, :, 64:65],
                    onesf[:, 0:32].rearrange("p (t h d) -> p t h d", t=8, h=HPG),
                )

            def g_qk(tq, fc, th):
                xts = xts_cur[0]
                dest = (qt if fc < 2 else kt)[fc % 2][tq]
                ps = ovpool.tile([128, 512], f32, name="fps", tag=borrow[0][th])
                for ec in range(4):
                    nc.tensor.matmul(
                        ps[:],
                        lhsT=wqk_sb[:, ec * 512 + fc * 128 : ec * 512 + (fc + 1) * 128],
                        rhs=xts[ec][:, th * 512 : (th + 1) * 512],
                        start=(ec == 0),
                        stop=(ec == 3),
                    )
                nc.vector.tensor_scalar_add(
                    dest[:, th * 512 : (th + 1) * 512], ps[:], bqk_sb[:, fc : fc + 1]
                )

            def g_v(tq, tb):
                xts = xts_cur[0]
                v_tile = vt[tq]
                vps = ovpool.tile(
                    [128, GQ], f32, name="fvps", tag=borrow[0][tb % 2]
                )
                for ec in range(4):
                    nc.tensor.matmul(
                        vps[:],
                        lhsT=xts[ec][:, tb * 128 : (tb + 1) * 128],
                        rhs=wv_sb[:, ec * GQ : (ec + 1) * GQ],
                        start=(ec == 0),
                        stop=False,
                    )
                nc.tensor.matmul(
                    vps[:], lhsT=ones_row[:], rhs=bv_sb[:], start=False, stop=True
                )
                nc.vector.tensor_copy(
                    v_tile[:, tb * VW : (tb + 1) * VW].rearrange(
                        "p (h d) -> p h d", h=HPG
                    )[:, :, 0:64],
                    vps.rearrange("p (h d) -> p h d", h=HPG),
                )

            osb_cur = [None]

            def g_wo(qq, tb):
                # out rows [qq*1024 + tb*128 ...]: contract both pair tiles
                if tb == 0 or tb == 4:
                    osb_cur[0] = opool.tile(
                        [128, 4 * 512], f32, name="osb", tag=f"osb{qq % 2}"
                    )
                out_sb = osb_cur[0]
                qqp, tbl = (2 * qq + tb // 4), tb % 4
                wops = ovpool.tile(
                    [128, 512], f32, name="fwops", tag=borrow[0][tb % 2]
                )
                for p in range(2):
                    nc.tensor.matmul(
                        wops[:],
                        lhsT=attt[qqp][p][:, tbl * 128 : (tbl + 1) * 128],
                        rhs=wo_sb[:, p * 512 : (p + 1) * 512],
                        start=(p == 0),
                        stop=(p == 1),
                    )
                nc.vector.tensor_copy(out_sb[:, tbl * 512 : (tbl + 1) * 512], wops[:])
                if tbl == 3:
                    half = tb // 4
                    nc.sync.dma_start(
                        out[
                            qq * 1024 + half * 512 : qq * 1024 + (half + 1) * 512, :
                        ].rearrange("(t p) c -> p t c", p=128),
                        out_sb.rearrange("p (t c) -> p t c", t=4),
                    )

            # Filler queues. p1 groups only read DMA'd inputs: safe to pull or
            # force-drain any time. wo groups read att tiles written by
            # epilogues still pending in the PV FIFO: pull-only (never
            # force-drained before the end).
            p1q = []  # (min_sweep, deadline_sweep, fn)
            woq = []  # (min_sweep, fn)

            def p1_groups(tq, min_sweep, deadline):
                p1q.append((min_sweep, deadline, lambda tq=tq: g_xload(tq)))
                for gi, fc in enumerate((0, 2, 1, 3)):
                    for th in range(2):
                        p1q.append(
                            (min_sweep, deadline,
                             lambda tq=tq, fc=fc, th=th: g_qk(tq, fc, th))
                        )
                for tb in range(8):
                    p1q.append(
                        (min_sweep, deadline, lambda tq=tq, tb=tb: g_v(tq, tb))
                    )

            def wo_groups(qq, min_sweep):
                # tb 0-3 read attt[2qq] (ready one sweep earlier than tb 4-7).
                # Defer to sweep >= 5: late sweeps are where the PE runs out of
                # projection filler while ACT paces.
                for tb in range(8):
                    woq.append(
                        (max(min_sweep - (1 if tb < 4 else 0), 5),
                         lambda qq=qq, tb=tb: g_wo(qq, tb))
                    )

            def pull_filler(sweep):
                if p1q and p1q[0][0] <= sweep:
                    p1q.pop(0)[2]()
                elif woq and woq[0][0] <= sweep:
                    woq.pop(0)[1]()

            def drain_p1(sweep):
                while p1q and p1q[0][1] <= sweep:
                    p1q.pop(0)[2]()

            def epilogue(qqp, pr, ov_e, ov_o):
                # ov rows 0:64 = head output [dh, QE], row 64 = softmax sums.
                # Copy out of PSUM first (frees the ov banks), then one
                # broadcast DMA + one reciprocal serve both heads.
                pair = attt[qqp][pr]
                uat = epool.tile([128, QE], f32, name="uat", tag="uat")
                nc.vector.tensor_copy(uat[0:DH, :], ov_e[0:DH, :])
                nc.vector.tensor_copy(uat[DH:128, :], ov_o[0:DH, :])
                den2 = epool.tile([128, QE], f32, name="den2", tag="den2")
                nc.vector.tensor_copy(den2[0:1, :], ov_e[64:65, :])
                nc.vector.tensor_copy(den2[DH : DH + 1, :], ov_o[64:65, :])
                sbc = epool.tile([128, QE], f32, name="sbc", tag="sbc")
                # issue on the ACT engine's DGE queue so this small
                # latency-critical DMA never queues behind bulk out-stores
                nc.scalar.dma_start(
                    sbc[:],
                    den2.rearrange("(a b) c -> a b c", a=2)[:, 0:1, :].to_broadcast(
                        [2, DH, QE]
                    ),
                )
                rbc = epool.tile([128, QE], f32, name="rbc", tag="rbc")
                nc.vector.reciprocal_approx_fast(out=rbc[:], in_=sbc[:])
                nc.vector.tensor_tensor(pair[0:DH, :], uat[0:DH, :], rbc[0:DH, :], ALU.mult)
                nc.vector.tensor_tensor(
                    pair[DH:128, :], uat[DH:128, :], rbc[DH:128, :], ALU.mult
                )

            # ---- software-pipelined attention ----
            stepc = [0]  # global st-buffer parity
            pend = []  # FIFO of deferred PV emitters; kept 2 slots deep

            def flush_slot(sweep, fill=True):
                if len(pend) > 2:
                    pend.pop(0)()
                    if fill:
                        pull_filler(sweep)

            def flush_all(sweep):
                while pend:
                    pend.pop(0)()

            unitc = [0]  # unit counter: ov parity; filler borrows other parity

            def att_unit(qq, pr):
                nkb = 4 * qq + 4
                par = unitc[0] % 2
                unitc[0] += 1
                borrow[0] = (f"ove{1 - par}", f"ovo{1 - par}")
                ov_e = ovpool.tile([65, QE], f32, name="ov_e", tag=f"ove{par}")
                ov_o = ovpool.tile([65, QE], f32, name="ov_o", tag=f"ovo{par}")
                qtile = qt[pr][qq // 2]
                qoff = (qq % 2) * 512
                for kb in range(nkb):
                    tqk, kbl = kb // 8, kb % 8
                    qs = max(0, (kb - 4 * qq) * 128)
                    st = stpool.tile(
                        [128, 2 * QE], f32, name="st", tag=f"st{stepc[0] % 2}"
                    )
                    stepc[0] += 1
                    for hh in range(2):
                        nc.tensor.matmul(
                            st[:, hh * 512 + qs : hh * 512 + 512],
                            lhsT=kt[pr][tqk][hh * 64 : hh * 64 + 64, kbl * 128 : (kbl + 1) * 128],
                            rhs=qtile[hh * 64 : hh * 64 + 64, qoff + qs : qoff + 512],
                            start=True,
                            stop=True,
                        )
                    if kb >= 4 * qq:  # diagonal: accumulate causal mask on PE
                        for hh in range(2):
                            nc.tensor.matmul(
                                st[:, hh * 512 + qs : hh * 512 + qs + 128],
                                lhsT=ident[:],
                                rhs=maskT[:],
                                start=False,
                                stop=True,
                                skip_group_check=True,
                            )
                    pt = ptpool.tile([128, 2 * QE], bf16, name="pt", tag="pt")
                    if qs == 0 or stepc[0] > 10:
                        # one wide inst; the gap [512, 512+qs) holds stale-but-
                        # bounded psum whose exp lands in pt cols PV never reads
                        nc.scalar.activation(
                            pt[:, qs : 2 * QE], st[:, qs : 2 * QE], AF.Exp,
                            bias=0.0, scale=SCALE,
                        )
                    else:
                        for hh in range(2):
                            nc.scalar.activation(
                                pt[:, hh * 512 + qs : hh * 512 + 512],
                                st[:, hh * 512 + qs : hh * 512 + 512],
                                AF.Exp, bias=0.0, scale=SCALE,
                            )
                    def pv(tqk=tqk, kbl=kbl, qs=qs, kb=kb, pt=pt, pr=pr,
                           ov_e=ov_e, ov_o=ov_o, nkb=nkb):
                        nc.tensor.matmul(
                            ov_e[:, qs:QE],
                            lhsT=vt[tqk][:, kbl * VW + 2 * pr * 65 : kbl * VW + (2 * pr + 1) * 65],
                            rhs=pt[:, qs:512],
                            start=(kb == 0),
                            stop=(kb == nkb - 1),
                            skip_group_check=True,
                        )
                        nc.tensor.matmul(
                            ov_o[:, qs:QE],
                            lhsT=vt[tqk][:, kbl * VW + (2 * pr + 1) * 65 : kbl * VW + (2 * pr + 2) * 65],
                            rhs=pt[:, 512 + qs : 1024],
                            start=(kb == 0),
                            stop=(kb == nkb - 1),
                            skip_group_check=True,
                        )

                    if kb == nkb - 1:
                        def pv_last(pv=pv, qq=qq, pr=pr, ov_e=ov_e, ov_o=ov_o):
                            pv()
                            epilogue(qq, pr, ov_e, ov_o)

                        pend.append(pv_last)
                    else:
                        pend.append(pv)
                    fill = (
                        (2 <= kb < nkb - 1) if qq <= 1
                        else (kb % 2 == 1 and 7 <= kb < nkb - 2)
                    )
                    flush_slot(qq, fill=fill)

            # ---- schedule ----
            # p1(0) up front; p1(1..3) + wo(0..3) interleaved as filler;
            # wo(3)'s second half at the end.
            p1_groups(0, min_sweep=-1, deadline=0)
            drain_p1(0)
            for tq in range(1, NTQ):
                p1_groups(tq, min_sweep=2 * (tq - 1), deadline=2 * tq)
            wo_groups(0, min_sweep=2)
            wo_groups(1, min_sweep=4)
            wo_groups(2, min_sweep=6)
            wo_groups(3, min_sweep=8)

            for qq in range(NQQ):
                drain_p1(qq)  # p1(tq) must be done before sweep 2tq
                for pr in range(2):
                    att_unit(qq, pr)
            flush_all(NQQ)
            drain_p1(10**9)
            while woq:
                woq.pop(0)[1]()

    nc.finalize()
    return nc


def _get_nc():
    if "nc" not in _CACHE:
        _CACHE["nc"] = _build_nc()
    return _CACHE["nc"]


def _make_in_maps(x, Wqkv, bqkv, Wo):
    in_maps = []
    for core in range(8):
        b, g = core // 2, core % 2
        qs, ks, vs = g * GQ, 512 + g * GQ, 1024 + g * GQ
        wqk_np = np.ascontiguousarray(
            np.concatenate([Wqkv[:, qs : qs + GQ], Wqkv[:, ks : ks + GQ]], axis=1)
        ).astype(BF16)
        bqk_np = np.ascontiguousarray(
            np.concatenate([bqkv[qs : qs + GQ], bqkv[ks : ks + GQ]]).reshape(4, 128).T
        )
        wv_np = np.ascontiguousarray(Wqkv[:, vs : vs + GQ]).astype(BF16)
        bv_np = np.ascontiguousarray(bqkv[vs : vs + GQ].reshape(1, GQ)).astype(BF16)
        wo_g = Wo[g * GQ : (g + 1) * GQ, :]
        # pair p block rows = heads 2p,2p+1 stacked = wo_g[p*128:(p+1)*128]
        wo_np = np.ascontiguousarray(
            np.concatenate([wo_g[0:128, :], wo_g[128:256, :]], axis=1)
        ).astype(BF16)
        in_maps.append(
            {
                "xT": np.ascontiguousarray(x[b].T).astype(BF16),
                "wqk": wqk_np,
                "bqk": bqk_np,
                "wv": wv_np,
                "bv": bv_np,
                "wo": wo_np,
            }
        )
    return in_maps


def kernel(x, Wqkv, bqkv, Wo, bo, **run_kwargs):
    from concourse.bass_utils import run_bass_kernel_spmd

    x = np.asarray(x, dtype=np.float32)
    Wqkv = np.asarray(Wqkv, dtype=np.float32)
    bqkv = np.asarray(bqkv, dtype=np.float32)
    Wo = np.asarray(Wo, dtype=np.float32)
    bo = np.asarray(bo, dtype=np.float32)

    nc = _get_nc()
    in_maps = _make_in_maps(x, Wqkv, bqkv, Wo)

    res = run_bass_kernel_spmd(nc, in_maps, core_ids=list(range(8)), **run_kwargs)
    _CACHE["last_results"] = res

    out = np.empty((B, S, E), dtype=np.float32)
    for b in range(B):
        out[b] = res.results[2 * b]["out"] + res.results[2 * b + 1]["out"] + bo
    return out
